# revision 4
# baseline (speedup 1.0000x reference)
"""Trainium2 Bass kernel for nn_CrossAttentionConditioner.

Reference computation (N=4096 edges, H=256 hidden, 4 heads, 64 graphs):
    K = key @ w_edge.T + b_edge ; V = value @ w_edge.T + b_edge
    Qh/Kh/Vh = in-proj of (query, K, V), 4 heads x 64
    block-diagonal (per-graph) softmax attention, out-proj, residual.

Strategy (data parallel over graphs, 8 cores):
  * Host folds w_edge into wk/wv (Wk_eff = wk @ w_edge), so the device sees
    plain projections from the raw inputs.
  * Graphs (45..80 edges each, sorted/contiguous) are bin-packed (FFD) into
    128-row bins; each core gets T bins = 128*T padded rows. Attention is
    computed per 128-row bin with an on-device segment-equality mask, so it
    is exact block-diagonal attention (softmax without max-subtraction:
    scores are tiny; masked entries are multiplied by 0 after exp).
  * Per core: feature-major projections (QhT/KhT d-major, Vh row-major),
    per (bin, head): scoresT = KhT.T-slice x QhT-slice, exp via ACT,
    mask via DVE, denom via matmul with ones column, attnT = Vh.T x expT,
    out-proj per head, per-partition recip scaling folded into a
    scalar_tensor_tensor accumulation chain seeded with query + b_out.
"""

import numpy as np
from contextlib import ExitStack

import concourse.bacc as bacc
import concourse.bass as bass
import concourse.tile as tile
from concourse import mybir
from concourse.bass_utils import run_bass_kernel_spmd

NCORES = 8
NBIN = 128           # rows per attention bin (= SBUF partition count)
H = 256              # hidden dim
ED = 128             # edge feature dim
NHEADS = 4
DH = H // NHEADS     # 64
F32 = mybir.dt.float32
AT = mybir.ActivationFunctionType
OP = mybir.AluOpType

_PROG_CACHE = {}


def _build_program(T: int, with_bias: bool):
    """Emit the per-core Bass/Tile program for T bins of 128 rows."""
    NPAD = T * NBIN
    NH2 = NPAD // 2  # half of the row dim, per-matmul N (fits one PSUM bank for T<=8)
    assert NH2 * 4 <= 2048, "N half-slab must fit a PSUM bank"

    nc = bacc.Bacc("TRN2", debug=False, enable_asserts=False)

    qT_d = nc.dram_tensor("qT", [H, NPAD], F32, kind="ExternalInput").ap()
    qrow_d = nc.dram_tensor("qrow", [NPAD, H], F32, kind="ExternalInput").ap()
    kT_d = nc.dram_tensor("kT", [ED, NPAD], F32, kind="ExternalInput").ap()
    vT_d = nc.dram_tensor("vT", [ED, NPAD], F32, kind="ExternalInput").ap()
    seg_d = nc.dram_tensor("seg", [NPAD, 1], F32, kind="ExternalInput").ap()
    wqT_d = nc.dram_tensor("wqT", [H, H], F32, kind="ExternalInput").ap()
    wkT_d = nc.dram_tensor("wkT", [ED, H], F32, kind="ExternalInput").ap()
    wvT_d = nc.dram_tensor("wvT", [ED, H], F32, kind="ExternalInput").ap()
    woT_d = nc.dram_tensor("woT", [H, H], F32, kind="ExternalInput").ap()
    if with_bias:
        bias_d = nc.dram_tensor("biasp", [1, 3 * H], F32, kind="ExternalInput").ap()
    out_d = nc.dram_tensor("out", [NPAD, H], F32, kind="ExternalOutput").ap()

    with tile.TileContext(nc) as tc, ExitStack() as ctx:
        singles = ctx.enter_context(tc.tile_pool(name="singles", bufs=1))
        work = ctx.enter_context(tc.tile_pool(name="work", bufs=3))
        psum = ctx.enter_context(tc.tile_pool(name="psum", bufs=8, space="PSUM"))

        # ---- resident weights / activations -------------------------------
        wq_sb = singles.tile([128, 2, H], F32)
        nc.sync.dma_start(out=wq_sb, in_=wqT_d.rearrange("(a p) o -> p a o", p=128))
        wk_sb = singles.tile([128, H], F32)
        nc.sync.dma_start(out=wk_sb, in_=wkT_d)
        wv_sb = singles.tile([128, H], F32)
        nc.sync.dma_start(out=wv_sb, in_=wvT_d)
        wo_sb = singles.tile([128, 2, H], F32)
        nc.sync.dma_start(out=wo_sb, in_=woT_d.rearrange("(a p) o -> p a o", p=128))

        kT_sb = singles.tile([128, NPAD], F32)
        nc.sync.dma_start(out=kT_sb, in_=kT_d)
        vT_sb = singles.tile([128, NPAD], F32)
        nc.sync.dma_start(out=vT_sb, in_=vT_d)
        qT_sb = singles.tile([128, 2, NPAD], F32)
        nc.sync.dma_start(out=qT_sb, in_=qT_d.rearrange("(a p) n -> p a n", p=128))
        qrow_sb = singles.tile([128, T, H], F32)
        nc.sync.dma_start(out=qrow_sb, in_=qrow_d.rearrange("(t p) o -> p t o", p=128))

        seg_col = singles.tile([128, T], F32)
        nc.sync.dma_start(out=seg_col, in_=seg_d.rearrange("(t p) o -> p (t o)", p=128))
        seg_bc = singles.tile([128, T, 128], F32)
        nc.sync.dma_start(
            out=seg_bc,
            in_=bass.AP(tensor=seg_d.tensor, offset=0, ap=[[0, 128], [128, T], [1, 128]]),
        )

        ones_col = singles.tile([128, 1], F32)
        nc.vector.memset(ones_col, 1.0)
        if with_bias:
            bias_sb = singles.tile([1, 3 * H], F32)
            nc.sync.dma_start(out=bias_sb, in_=bias_d)
            ones_row = singles.tile([1, NH2], F32)
            nc.vector.memset(ones_row, 1.0)

        QhT_sb = singles.tile([128, 2, NPAD], F32)
        KhT_sb = singles.tile([128, 2, NPAD], F32)
        Vh_sb = singles.tile([128, T, H], F32)
        mask_sb = singles.tile([128, T, 128], F32)

        # ---- phase A: projections ----------------------------------------
        # QhT[o, n] = sum_h wq[o, h] qT[h, n]  (d-major), same for KhT.
        copy_flip = 0

        def copy_out(dst_ap, src_ap):
            nonlocal copy_flip
            if copy_flip % 2 == 0:
                nc.vector.tensor_copy(out=dst_ap, in_=src_ap)
            else:
                nc.scalar.copy(out=dst_ap, in_=src_ap)
            copy_flip += 1

        for src_sb, w_sb, nkt, dst_sb, boff in (
            (qT_sb, wq_sb, 2, QhT_sb, 0),
            (kT_sb, wk_sb, 1, KhT_sb, H),
        ):
            for ot in range(2):
                for h2 in range(2):
                    ps = psum.tile([128, NH2], F32, tag="ps")
                    for kt in range(nkt):
                        lhsT = w_sb[:, kt, ot * 128:(ot + 1) * 128] if nkt == 2 else \
                            w_sb[:, ot * 128:(ot + 1) * 128]
                        rhs = src_sb[:, kt, h2 * NH2:(h2 + 1) * NH2] if nkt == 2 else \
                            src_sb[:, h2 * NH2:(h2 + 1) * NH2]
                        nc.tensor.matmul(
                            ps, lhsT, rhs,
                            start=(kt == 0),
                            stop=(kt == nkt - 1 and not with_bias),
                        )
                    if with_bias:
                        nc.tensor.matmul(
                            ps,
                            bias_sb[:, boff + ot * 128: boff + (ot + 1) * 128],
                            ones_row,
                            start=False, stop=True,
                        )
                    copy_out(dst_sb[:, ot, h2 * NH2:(h2 + 1) * NH2], ps)

        # Vh[n, d] = sum_h vT[h, n] WvT[h, d]  (row-major)
        for t in range(T):
            ps = psum.tile([128, H], F32, tag="ps")
            nc.tensor.matmul(
                ps, vT_sb[:, t * 128:(t + 1) * 128], wv_sb,
                start=True, stop=not with_bias,
            )
            if with_bias:
                nc.tensor.matmul(
                    ps, ones_row[:, :128], bias_sb[:, 2 * H:3 * H],
                    start=False, stop=True,
                )
            copy_out(Vh_sb[:, t, :], ps)

        # ---- masks: mask[t][j, i] = (seg[t*128+j] == seg[t*128+i]) --------
        for t in range(T):
            nc.vector.tensor_scalar(
                out=mask_sb[:, t, :], in0=seg_bc[:, t, :],
                scalar1=seg_col[:, t:t + 1], scalar2=None, op0=OP.is_equal,
            )

        # ---- phase B: per-bin attention ----------------------------------
        for t in range(T):
            recip_t = work.tile([128, NHEADS], F32, tag="recip")
            pair_sb = []
            for hp in range(2):
                attn_ps = psum.tile([128, 128], F32, tag="ps")
                for hh in range(2):
                    h = hp * 2 + hh
                    pslc = slice(64 * hh, 64 * hh + 64)
                    sc_ps = psum.tile([128, 128], F32, tag="ps")
                    nc.tensor.matmul(
                        sc_ps,
                        KhT_sb[pslc, hp, t * 128:(t + 1) * 128],
                        QhT_sb[pslc, hp, t * 128:(t + 1) * 128],
                        start=True, stop=True,
                    )
                    expT = work.tile([128, 128], F32, tag="expT")
                    nc.scalar.activation(expT, sc_ps, AT.Exp, scale=0.125)
                    nc.vector.tensor_mul(expT, expT, mask_sb[:, t, :])
                    den_ps = psum.tile([128, 1], F32, tag="ps")
                    nc.tensor.matmul(den_ps, expT, ones_col, start=True, stop=True)
                    nc.vector.reciprocal(recip_t[:, h:h + 1], den_ps)
                    nc.tensor.matmul(
                        attn_ps[pslc, :],
                        Vh_sb[:, t, 64 * h:64 * h + 64],
                        expT,
                        start=True, stop=True,
                    )
                sb = work.tile([128, 128], F32, tag="pair")
                copy_out(sb, attn_ps)
                pair_sb.append(sb)

            acc = qrow_sb[:, t, :]
            for h in range(NHEADS):
                hp, hh = h // 2, h % 2
                pslc = slice(64 * hh, 64 * hh + 64)
                proj_ps = psum.tile([128, H], F32, tag="ps")
                nc.tensor.matmul(
                    proj_ps, pair_sb[hp][pslc, :], wo_sb[pslc, hp, :],
                    start=True, stop=True,
                )
                acc_new = work.tile([128, H], F32, tag="acc")
                nc.vector.scalar_tensor_tensor(
                    out=acc_new, in0=proj_ps, scalar=recip_t[:, h:h + 1],
                    in1=acc, op0=OP.mult, op1=OP.add,
                )
                acc = acc_new
            nc.sync.dma_start(out=out_d[t * 128:(t + 1) * 128, :], in_=acc)

    nc.compile()
    return nc


def _plan(seg: np.ndarray):
    """FFD bin-pack whole graphs into 128-row bins; chunk bins over 8 cores.

    Returns (T, bins) where bins is a list of lists of (start, end) row
    ranges (one per graph), padded with empty bins to a multiple of NCORES.
    """
    ngraph = int(seg.max()) + 1 if seg.size else 0
    sizes = np.bincount(seg, minlength=ngraph)
    starts = np.concatenate([[0], np.cumsum(sizes)])
    assert sizes.max() <= NBIN, (
        f"graph with {sizes.max()} edges exceeds the {NBIN}-row attention bin"
    )
    order = np.argsort(-sizes, kind="stable")
    bins = []  # [fill, [graph ids]]
    for g in order:
        s = int(sizes[g])
        if s == 0:
            continue
        for b in bins:
            if b[0] + s <= NBIN:
                b[0] += s
                b[1].append(int(g))
                break
        else:
            bins.append([s, [int(g)]])
    while len(bins) % NCORES:
        bins.append([0, []])
    T = len(bins) // NCORES
    ranges = [[(int(starts[g]), int(starts[g + 1])) for g in b[1]] for b in bins]
    return T, ranges


def kernel(query, key, value, edge_graph_index,
           w_edge, b_edge, w_in, b_in, w_out, b_out,
           _trace=False):
    query = np.ascontiguousarray(np.asarray(query, dtype=np.float32))
    key = np.ascontiguousarray(np.asarray(key, dtype=np.float32))
    value = np.ascontiguousarray(np.asarray(value, dtype=np.float32))
    seg = np.asarray(edge_graph_index).astype(np.int64)
    w_edge = np.asarray(w_edge, dtype=np.float32)
    b_edge = np.asarray(b_edge, dtype=np.float32)
    w_in = np.asarray(w_in, dtype=np.float32)
    b_in = np.asarray(b_in, dtype=np.float32)
    w_out = np.asarray(w_out, dtype=np.float32)
    b_out = np.asarray(b_out, dtype=np.float32)

    N = query.shape[0]

    # ---- host-side weight folding ------------------------------------
    wq, wk, wv = np.split(w_in, 3, axis=0)
    bq, bk, bv = np.split(b_in, 3)
    wqT = np.ascontiguousarray(wq.T)                    # [H, H]
    wkT = np.ascontiguousarray((wk @ w_edge).T)         # [ED, H]
    wvT = np.ascontiguousarray((wv @ w_edge).T)         # [ED, H]
    bk_eff = wk @ b_edge + bk
    bv_eff = wv @ b_edge + bv
    woT = np.ascontiguousarray(w_out.T)                 # [H, H]
    with_bias = bool(
        np.abs(bq).max() > 0 or np.abs(bk_eff).max() > 0 or np.abs(bv_eff).max() > 0
    )

    T, bin_ranges = _plan(seg)
    NPAD = T * NBIN

    ck = (T, with_bias)
    if ck not in _PROG_CACHE:
        _PROG_CACHE[ck] = _build_program(T, with_bias)
    nc = _PROG_CACHE[ck]

    # ---- pack per-core inputs ----------------------------------------
    in_maps = []
    rowmaps = []
    bias_pack = np.concatenate([bq, bk_eff, bv_eff]).reshape(1, 3 * H)
    bias_pack = np.ascontiguousarray(bias_pack, dtype=np.float32)
    for c in range(NCORES):
        qc = np.zeros((NPAD, H), np.float32)
        kc = np.zeros((NPAD, ED), np.float32)
        vc = np.zeros((NPAD, ED), np.float32)
        sc = np.full((NPAD, 1), -1.0, np.float32)
        rowmap = np.full(NPAD, -1, np.int64)
        for bi, ranges in enumerate(bin_ranges[c * T:(c + 1) * T]):
            off = bi * NBIN
            for a, b in ranges:
                n = b - a
                qc[off:off + n] = query[a:b]
                kc[off:off + n] = key[a:b]
                vc[off:off + n] = value[a:b]
                sc[off:off + n, 0] = seg[a:b]
                rowmap[off:off + n] = np.arange(a, b)
                off += n
        rowmaps.append(rowmap)
        im = {
            "qT": np.ascontiguousarray(qc.T),
            "qrow": np.ascontiguousarray(qc + b_out[None, :]),
            "kT": np.ascontiguousarray(kc.T),
            "vT": np.ascontiguousarray(vc.T),
            "seg": sc,
            "wqT": wqT,
            "wkT": wkT,
            "wvT": wvT,
            "woT": woT,
        }
        if with_bias:
            im["biasp"] = bias_pack
        in_maps.append(im)

    res = run_bass_kernel_spmd(
        nc, in_maps, core_ids=list(range(NCORES)), trace=_trace,
        **({"trace_cores": list(range(NCORES))} if _trace else {}),
    )

    out_full = np.zeros((N, H), np.float32)
    for c in range(NCORES):
        oc = res.results[c]["out"]
        valid = rowmaps[c] >= 0
        out_full[rowmaps[c][valid]] = oc[valid]

    if _trace:
        return out_full, res
    return out_full
